# revision 16
# baseline (speedup 1.0000x reference)
"""MoD block (router top-k mask + causal MHA + SwiGLU FFN) on 8 TRN2 cores.

Sharding: batch row b = core//4; each core owns two 256-token chunks
(j, 7-j) of its row (causally balanced).  Cores are fully independent
(no collectives): each computes LN1 + K/V projections for the whole row,
attention + FFN only for its own 512 tokens.  Causal structure is
data-driven (additive mask inputs) so all 8 cores run one SPMD program.

Matmuls run in float32r (full-rate fp32 on the PE); the router path is
fp32 on the vector engine so the top-k mask matches the reference's
exact selection.
"""
import sys

sys.path.insert(0, "/opt/trn_rl_repo")

import numpy as np

import concourse.bass as bass
import concourse.mybir as mybir
import concourse.tile as tile
from concourse import bacc
from concourse.bass_utils import run_bass_kernel_spmd

F32 = mybir.dt.float32
F32R = mybir.dt.float32r
AF = mybir.ActivationFunctionType
OP = mybir.AluOpType

B, S, H, FF = 2, 2048, 1024, 4096
NH, HD = 16, 64
CAP = S // 2          # capacity = 1024
CH = 256              # chunk size
P = 128
NEG = -60000.0        # additive causal-mask value (exp -> 0)
EPS = 1e-6

NKA, NKB = 8, 16      # padded k-tiles for slot A / slot B
NMT = NKA + NKB       # mask tiles per core


import os
_PHASE = int(os.environ.get("KPHASE", "5"))


def _emit(nc, tc, t_in, t_out):
    xr_d = t_in["xr"]          # [S, H] full batch row
    xo_d = t_in["xo"]          # [512, H] own tokens (chunk j ++ chunk 7-j)
    msk_d = t_in["msk"]        # [P, NMT, CH] additive causal masks
    out_d = t_out["out"]       # [512, H]
    lg_d = t_out["lg"]         # [S] router logits (full row)

    xr = xr_d.ap().rearrange("(n p) h -> p n h", p=P)      # [128, 16, H]
    xo = xo_d.ap().rearrange("(n p) h -> p n h", p=P)      # [128, 4, H]
    out_ap = out_d.ap().rearrange("(n p) h -> p n h", p=P)

    singles = tc.alloc_tile_pool(name="singles", bufs=1)
    dram = tc.alloc_tile_pool(name="dram", bufs=1, space="DRAM")

    # ---- constants (small, whole-program) ----
    ident = singles.tile([P, P], F32)
    nc.sync.dma_start(ident, t_in["ident"].ap())
    ident_r = singles.tile([P, P], F32R)
    nc.vector.tensor_copy(ident_r, ident)
    ones64 = singles.tile([P, 64], F32)
    nc.vector.memset(ones64, 1.0)
    epst = singles.tile([P, 1], F32)
    nc.vector.memset(epst, EPS)
    wrb = singles.tile([P, H], F32)  # router weights (pre-broadcast on host)
    nc.sync.dma_start(wrb, t_in["wr"].ap())
    lcol = singles.tile([P, 16], F32)   # full-row logits, column-major
    locol = singles.tile([P, 4], F32)   # own logits
    maskc = singles.tile([P, 4], F32)   # own routing mask

    ktsp = dram.tile([P, 8, S], F32R)   # K^T spill (head-pair packed)
    l_rt = dram.tile([16, P], F32)      # logits roundtrip

    def layernorm(xt, xnt, pool):
        """xt, xnt: [P, H] fp32 SBUF; writes xnt = (x-mu)*rstd.  May alias."""
        st = pool.tile([P, 2, 6], F32, tag="ln_st")
        mv = pool.tile([P, 2], F32, tag="ln_mv")
        nc.vector.bn_stats(st[:, 0], xt[:, 0:512])
        nc.vector.bn_stats(st[:, 1], xt[:, 512:1024])
        nc.vector.bn_aggr(mv, st)
        rs = pool.tile([P, 1], F32, tag="ln_rs")
        nc.scalar.activation(rs, mv[:, 1:2], AF.Sqrt, bias=epst, scale=1.0)
        nc.vector.reciprocal(rs, rs)
        nc.vector.tensor_scalar(
            xnt, xt, mv[:, 0:1], rs, op0=OP.subtract, op1=OP.mult
        )

    # x_own doubles as x2 (residual updated in place after attention)
    xob = tc.alloc_tile_pool(name="xob", bufs=1)
    x_own = xob.tile([P, 4, H], F32)

    # ============ band: v_aug (P1..P2) ============
    vb = tc.alloc_tile_pool(name="vb", bufs=1)
    v_aug = vb.tile([P, 16, NH, HD + 1], F32R)   # V (natural) + ones col
    nc.vector.tensor_copy(
        v_aug[:, :, :, HD : HD + 1],
        ones64[:, 0:1].to_broadcast([P, 16, NH, 1]),
    )

    # ================= P1: full row: LN1, router, K/V proj =================
    with (
        tc.tile_pool(name="p1w", bufs=1) as p1w,
        tc.tile_pool(name="p1", bufs=2) as p1,
        tc.tile_pool(name="p1s", bufs=2) as p1s,
        tc.tile_pool(name="p1ps", bufs=2, space="PSUM") as p1ps,
        tc.tile_pool(name="p1pt", bufs=2, space="PSUM") as p1pt,
    ):
        wkT = p1w.tile([P, 8, H], F32R)
        wvT = p1w.tile([P, 8, H], F32R)
        nc.sync.dma_start(
            wkT, t_in["wkT"].ap().rearrange("(a p) o -> p a o", p=P).bitcast(F32R)
        )
        nc.sync.dma_start(
            wvT, t_in["wvT"].ap().rearrange("(a p) o -> p a o", p=P).bitcast(F32R)
        )

        for sc in range(8):  # 256-token chunks of the full row
            xt = p1.tile([P, 2, H], F32, tag="xt")
            nc.sync.dma_start(xt, xr[:, 2 * sc : 2 * sc + 2, :])
            xnT = p1.tile([P, 8, CH], F32R, tag="xnT")
            for t in range(2):
                # router logits for these 128 tokens (reads raw x first)
                scr = p1s.tile([P, H], F32, tag="rt_scr")
                nc.vector.tensor_tensor(scr, xt[:, t], wrb, OP.mult)
                nc.vector.reduce_sum(
                    out=lcol[:, 2 * sc + t : 2 * sc + t + 1],
                    in_=scr,
                    axis=mybir.AxisListType.X,
                )
                layernorm(xt[:, t], xt[:, t], p1s)  # in-place
                for ht in range(8):
                    pst = p1pt.tile([P, P], F32, tag="tp")
                    nc.tensor.transpose(pst, xt[:, t, P * ht : P * (ht + 1)], ident)
                    nc.vector.tensor_copy(xnT[:, ht, P * t : P * (t + 1)], pst)
            # K projection (head-pair packed rows), spill to DRAM
            for op_ in range(8):
                ps = p1ps.tile([P, 512], F32, tag="kv")
                for ht in range(8):
                    nc.tensor.matmul(
                        ps[:, 0:CH],
                        wkT[:, ht, P * op_ : P * (op_ + 1)],
                        xnT[:, ht, :],
                        start=(ht == 0),
                        stop=(ht == 7),
                    )
                ks = p1s.tile([P, CH], F32R, tag="kstg")
                nc.vector.tensor_copy(ks, ps[:, 0:CH])
                nc.sync.dma_start(ktsp[:, op_, CH * sc : CH * (sc + 1)], ks)
            # V projection (natural layout) into v_aug
            for t in range(2):
                for oc in range(2):
                    ps = p1ps.tile([P, 512], F32, tag="kv")
                    for ht in range(8):
                        nc.tensor.matmul(
                            ps,
                            xnT[:, ht, P * t : P * (t + 1)],
                            wvT[:, ht, 512 * oc : 512 * (oc + 1)],
                            start=(ht == 0),
                            stop=(ht == 7),
                        )
                    nc.vector.tensor_copy(
                        v_aug[:, 2 * sc + t, 8 * oc : 8 * (oc + 1), 0:HD], ps
                    )

    if _PHASE < 2:
        xob.release()
        vb.release()
        singles.release()
        dram.release()
        return

    # ============ band: qT (P1b..P2) ============
    qb = tc.alloc_tile_pool(name="qb", bufs=1)
    qT = qb.tile([P, 8, 512], F32R)              # Q^T (head-pair packed)

    # ================= P1b: own tokens: LN1, router, Q proj =================
    with (
        tc.tile_pool(name="p2w", bufs=1) as p2w,
        tc.tile_pool(name="p2", bufs=1) as p2,
        tc.tile_pool(name="p2s", bufs=2) as p2s,
        tc.tile_pool(name="p2ps", bufs=2, space="PSUM") as p2ps,
        tc.tile_pool(name="p2pt", bufs=2, space="PSUM") as p2pt,
    ):
        wqT = p2w.tile([P, 8, H], F32R)
        nc.sync.dma_start(
            wqT, t_in["wqT"].ap().rearrange("(a p) o -> p a o", p=P).bitcast(F32R)
        )
        xnoT = p2w.tile([P, 8, 512], F32R)
        nc.sync.dma_start(x_own, xo)
        xno = p2.tile([P, 4, H], F32, tag="xno")
        for t in range(4):
            scr = p2s.tile([P, H], F32, tag="rt_scr")
            nc.vector.tensor_tensor(scr, x_own[:, t], wrb, OP.mult)
            nc.vector.reduce_sum(
                out=locol[:, t : t + 1], in_=scr, axis=mybir.AxisListType.X
            )
            layernorm(x_own[:, t], xno[:, t], p2s)
            for ht in range(8):
                pst = p2pt.tile([P, P], F32, tag="tp")
                nc.tensor.transpose(pst, xno[:, t, P * ht : P * (ht + 1)], ident)
                nc.vector.tensor_copy(xnoT[:, ht, P * t : P * (t + 1)], pst)
        for op_ in range(8):
            ps = p2ps.tile([P, 512], F32, tag="q")
            for ht in range(8):
                nc.tensor.matmul(
                    ps,
                    wqT[:, ht, P * op_ : P * (op_ + 1)],
                    xnoT[:, ht, :],
                    start=(ht == 0),
                    stop=(ht == 7),
                )
            # fold in attention scale 1/sqrt(hd)
            nc.vector.tensor_scalar(
                qT[:, op_, :], ps, float(HD) ** -0.5, None, op0=OP.mult
            )

        # ---- router mask (rank of own logits within full row) ----
        ltp = p2pt.tile([16, P], F32, tag="ltp")
        nc.tensor.transpose(ltp, lcol, ident)
        lrow = p2s.tile([16, P], F32, tag="lrow")
        nc.vector.tensor_copy(lrow, ltp)
        nc.sync.dma_start(l_rt, lrow)
        nc.sync.dma_start(lg_d.ap().rearrange("(n p) -> n p", p=P), lrow)
        lbc = p2w.tile([P, S], F32)
        nc.sync.dma_start(
            lbc, bass.AP(tensor=l_rt.tensor, offset=l_rt.offset, ap=[[0, P], [1, S]])
        )
        for t in range(4):
            cmp = p2s.tile([P, S], F32, tag="cmp")
            rk = p2s.tile([P, 1], F32, tag="rk")
            nc.vector.tensor_scalar(
                cmp, lbc, locol[:, t : t + 1], None, op0=OP.is_gt
            )
            nc.vector.reduce_sum(out=rk, in_=cmp, axis=mybir.AxisListType.X)
            nc.vector.tensor_scalar(
                maskc[:, t : t + 1], rk, CAP - 0.5, None, op0=OP.is_le
            )

    if _PHASE < 3:
        qb.release()
        vb.release()
        xob.release()
        singles.release()
        dram.release()
        return

    # ================= P2: attention (+P3 inside ucT scope) =================
    ub = tc.alloc_tile_pool(name="ub", bufs=1)
    ucT = ub.tile([64, NH, 512], F32R)           # per-head U^T, denom-scaled
    with (
        tc.tile_pool(name="a_m", bufs=1) as a_m,
        tc.tile_pool(name="a_u", bufs=4, space="PSUM") as a_u,
        tc.tile_pool(name="a_s", bufs=3, space="PSUM") as a_s,
        tc.tile_pool(name="a_r", bufs=1, space="PSUM") as a_r,
        tc.tile_pool(name="a_sb", bufs=3) as a_sb,
        tc.tile_pool(name="a_kt", bufs=3) as a_kt,
    ):
        masks = a_m.tile([P, NMT, CH], F32R)
        nc.sync.dma_start(masks, msk_d.ap().bitcast(F32R))
        for slot in range(2):
            nk = NKA if slot == 0 else NKB
            moff = 0 if slot == 0 else NKA
            qsl = slice(CH * slot, CH * (slot + 1))
            for pp in range(4):  # head passes of 4
                us = [
                    a_u.tile([HD + 1, CH], F32, tag="u", name=f"u{i}")
                    for i in range(4)
                ]
                for kt in range(nk):
                    kts = a_kt.tile([P, 2, P], F32R, tag="kt")
                    nc.sync.dma_start(
                        kts, ktsp[:, 2 * pp : 2 * pp + 2, P * kt : P * (kt + 1)]
                    )
                    for h4 in range(4):
                        h = 4 * pp + h4
                        base = (h % 2) * 64
                        sps = a_s.tile([P, CH], F32, tag="s")
                        nc.tensor.matmul(
                            sps,
                            kts[base : base + 64, h4 // 2, :],
                            qT[base : base + 64, h // 2, qsl],
                            start=True,
                            stop=False,
                        )
                        nc.tensor.matmul(
                            sps,
                            ident_r,
                            masks[:, moff + kt, :],
                            start=False,
                            stop=True,
                        )
                        e = a_sb.tile([P, CH], F32R, tag="e")
                        nc.scalar.activation(e, sps, AF.Exp)
                        nc.tensor.matmul(
                            us[h4],
                            v_aug[:, kt, h, :],
                            e,
                            start=(kt == 0),
                            stop=(kt == nk - 1),
                        )
                for h4 in range(4):
                    h = 4 * pp + h4
                    scr = a_sb.tile([P, CH], F32, tag="dscr")
                    nc.vector.reciprocal(scr[64:65, :], us[h4][64:65, :])
                    rps = a_r.tile([64, CH], F32, tag="r")
                    nc.tensor.matmul(
                        rps, ones64[64:65, :], scr[64:65, :], start=True, stop=True
                    )
                    rsb = a_sb.tile([64, CH], F32, tag="rsb")
                    nc.vector.tensor_copy(rsb, rps)
                    nc.vector.tensor_tensor(
                        ucT[:, h, qsl], us[h4][0:64, :], rsb, OP.mult
                    )
    # ================= P3: output projection + residual (into x_own) ===
    with (
        tc.tile_pool(name="o_w", bufs=2) as o_w,
        tc.tile_pool(name="o_ps", bufs=2, space="PSUM") as o_ps,
        tc.tile_pool(name="o_sb", bufs=2) as o_sb,
    ):
        for oc in range(2):
            wos = o_w.tile([64, NH, 512], F32R, tag="wo")
            nc.sync.dma_start(
                wos,
                t_in["woT"]
                .ap()
                .rearrange("(a d) o -> d a o", d=64)[:, :, 512 * oc : 512 * (oc + 1)]
                .bitcast(F32R),
            )
            for st in range(4):
                ps = o_ps.tile([P, 512], F32, tag="o")
                for h in range(NH):
                    nc.tensor.matmul(
                        ps,
                        ucT[:, h, P * st : P * (st + 1)],
                        wos[:, h, :],
                        start=(h == 0),
                        stop=(h == NH - 1),
                    )
                tmp = o_sb.tile([P, 512], F32, tag="otmp")
                nc.vector.tensor_scalar(
                    tmp, ps, maskc[:, st : st + 1], None, op0=OP.mult
                )
                nc.vector.tensor_add(
                    x_own[:, st, 512 * oc : 512 * (oc + 1)],
                    tmp,
                    x_own[:, st, 512 * oc : 512 * (oc + 1)],
                )
    ub.release()
    qb.release()
    vb.release()
    x2 = x_own  # residual stream, updated in place

    if _PHASE < 5:
        xob.release()
        singles.release()
        dram.release()
        return

    # ================= P4: LN2 + SwiGLU FFN =================
    with (
        tc.tile_pool(name="f_h", bufs=1) as f_h,
        tc.tile_pool(name="f_s", bufs=2) as f_s,
        tc.tile_pool(name="f_ps", bufs=2, space="PSUM") as f_ps,
    ):
        hsb = f_h.tile([P, 32, 512], F32R)
        with (
            tc.tile_pool(name="f_x", bufs=1) as f_x,
            tc.tile_pool(name="f_w", bufs=2) as f_w,
            tc.tile_pool(name="f_pt", bufs=2, space="PSUM") as f_pt,
        ):
            xn2T = f_x.tile([P, 8, 512], F32R)
            xn2 = f_x.tile([P, 4, H], F32)
            for t in range(4):
                layernorm(x2[:, t], xn2[:, t], f_s)
                for ht in range(8):
                    pst = f_pt.tile([P, P], F32, tag="tp")
                    nc.tensor.transpose(pst, xn2[:, t, P * ht : P * (ht + 1)], ident)
                    nc.vector.tensor_copy(xn2T[:, ht, P * t : P * (t + 1)], pst)

            for fg in range(8):  # 512-wide f groups
                wgs = f_w.tile([P, 8, 512], F32R, tag="wg")
                wus = f_w.tile([P, 8, 512], F32R, tag="wu")
                for wt, nm in [(wgs, "wgT"), (wus, "wuT")]:
                    nc.sync.dma_start(
                        wt,
                        t_in[nm]
                        .ap()
                        .rearrange("(a p) f -> p a f", p=P)[
                            :, :, 512 * fg : 512 * (fg + 1)
                        ]
                        .bitcast(F32R),
                    )
                for fc in range(4):
                    gps = f_ps.tile([P, 512], F32, tag="g")
                    ups = f_ps.tile([P, 512], F32, tag="u")
                    for ht in range(8):
                        nc.tensor.matmul(
                            gps,
                            wgs[:, ht, P * fc : P * (fc + 1)],
                            xn2T[:, ht, :],
                            start=(ht == 0),
                            stop=(ht == 7),
                        )
                    for ht in range(8):
                        nc.tensor.matmul(
                            ups,
                            wus[:, ht, P * fc : P * (fc + 1)],
                            xn2T[:, ht, :],
                            start=(ht == 0),
                            stop=(ht == 7),
                        )
    # silu(g)*up as up*sigmoid(g)*g (sim has no Silu LUT)
                    gs = f_s.tile([P, 512], F32, tag="gsig")
                    nc.scalar.activation(gs, gps, AF.Sigmoid)
                    gu = f_s.tile([P, 512], F32, tag="gu")
                    nc.vector.tensor_tensor(gu, ups, gs, OP.mult)
                    nc.vector.tensor_tensor(hsb[:, 4 * fg + fc, :], gu, gps, OP.mult)

        with tc.tile_pool(name="f_wd", bufs=2) as f_wd:
            wdr = t_in["wdT"].ap().rearrange("(a p) o -> p a o", p=P)
            for oc in range(2):
                wh = []
                for half in range(2):
                    w = f_wd.tile([P, 16, 512], F32R, tag="wd", name=f"wd{half}")
                    nc.sync.dma_start(
                        w,
                        wdr[:, 16 * half : 16 * (half + 1),
                            512 * oc : 512 * (oc + 1)].bitcast(F32R),
                    )
                    wh.append(w)
                for st in range(4):
                    ps = f_ps.tile([P, 512], F32, tag="d")
                    for ft in range(32):
                        nc.tensor.matmul(
                            ps,
                            hsb[:, ft, P * st : P * (st + 1)],
                            wh[ft // 16][:, ft % 16, :],
                            start=(ft == 0),
                            stop=(ft == 31),
                        )
                    tmp = f_s.tile([P, 512], F32, tag="ftmp")
                    nc.vector.tensor_scalar(
                        tmp, ps, maskc[:, st : st + 1], None, op0=OP.mult
                    )
                    o = f_s.tile([P, 512], F32, tag="fout")
                    nc.vector.tensor_add(
                        o, tmp, x2[:, st, 512 * oc : 512 * (oc + 1)]
                    )
                    nc.sync.dma_start(out_ap[:, st, 512 * oc : 512 * (oc + 1)], o)
    xob.release()
    singles.release()
    dram.release()


_CACHED = None
_LAST_IN_MAPS = None


def _build():
    global _CACHED
    if _CACHED is not None:
        return _CACHED
    nc = bacc.Bacc("TRN2", target_bir_lowering=False, debug=False, num_devices=8)
    t_in = {
        "xr": nc.dram_tensor("xr", [S, H], F32, kind="ExternalInput"),
        "xo": nc.dram_tensor("xo", [512, H], F32, kind="ExternalInput"),
        "msk": nc.dram_tensor("msk", [P, NMT, CH], F32, kind="ExternalInput"),
        "wqT": nc.dram_tensor("wqT", [H, H], F32, kind="ExternalInput"),
        "wkT": nc.dram_tensor("wkT", [H, H], F32, kind="ExternalInput"),
        "wvT": nc.dram_tensor("wvT", [H, H], F32, kind="ExternalInput"),
        "woT": nc.dram_tensor("woT", [H, H], F32, kind="ExternalInput"),
        "wgT": nc.dram_tensor("wgT", [H, FF], F32, kind="ExternalInput"),
        "wuT": nc.dram_tensor("wuT", [H, FF], F32, kind="ExternalInput"),
        "wdT": nc.dram_tensor("wdT", [FF, H], F32, kind="ExternalInput"),
        "wr": nc.dram_tensor("wr", [P, H], F32, kind="ExternalInput"),
        "ident": nc.dram_tensor("ident", [P, P], F32, kind="ExternalInput"),
    }
    t_out = {
        "out": nc.dram_tensor("out", [512, H], F32, kind="ExternalOutput"),
        "lg": nc.dram_tensor("lg", [S], F32, kind="ExternalOutput"),
    }
    with tile.TileContext(nc) as tc:
        _emit(nc, tc, t_in, t_out)
    nc.compile()
    _CACHED = nc
    return nc


def _host_masks(j):
    """Additive causal masks for core chunk-pair (j, 7-j): [P, NMT, CH]."""
    m = np.zeros((P, NMT, CH), np.float32)
    p = np.arange(P)[:, None]
    f = np.arange(CH)[None, :]
    for kt in range(NKA):
        q0 = CH * j
        m[:, kt, :] = np.where(P * kt + p <= q0 + f, 0.0, NEG)
    for kt in range(NKB):
        q0 = CH * (7 - j)
        m[:, NKA + kt, :] = np.where(P * kt + p <= q0 + f, 0.0, NEG)
    return m


def kernel(
    x, w_router, ln1_g, ln1_b, wq, wk, wv, wo, ln2_g, ln2_b, w_gate, w_up, w_down
):
    x = np.asarray(x, np.float32)
    assert np.allclose(np.asarray(ln1_b), 0) and np.allclose(np.asarray(ln2_b), 0), (
        "nonzero LayerNorm bias not supported"
    )
    g1 = np.asarray(ln1_g, np.float32)
    g2 = np.asarray(ln2_g, np.float32)
    wqT = np.ascontiguousarray((np.asarray(wq) * g1[None, :]).T, np.float32)
    wkT = np.ascontiguousarray((np.asarray(wk) * g1[None, :]).T, np.float32)
    wvT = np.ascontiguousarray((np.asarray(wv) * g1[None, :]).T, np.float32)
    woT = np.ascontiguousarray(np.asarray(wo).T, np.float32)
    wgT = np.ascontiguousarray((np.asarray(w_gate) * g2[None, :]).T, np.float32)
    wuT = np.ascontiguousarray((np.asarray(w_up) * g2[None, :]).T, np.float32)
    wdT = np.ascontiguousarray(np.asarray(w_down).T, np.float32)
    wr = np.ascontiguousarray(np.broadcast_to(np.asarray(w_router, np.float32).reshape(1, H), (P, H)))

    nc = _build()
    in_maps = []
    for c in range(8):
        b, j = c // 4, c % 4
        xo = np.ascontiguousarray(
            np.concatenate(
                [x[b, CH * j : CH * (j + 1)], x[b, CH * (7 - j) : CH * (8 - j)]], 0
            )
        )
        in_maps.append(
            {
                "xr": np.ascontiguousarray(x[b]),
                "xo": xo,
                "msk": _host_masks(j),
                "wqT": wqT, "wkT": wkT, "wvT": wvT, "woT": woT,
                "wgT": wgT, "wuT": wuT, "wdT": wdT, "wr": wr,
                "ident": np.eye(P, dtype=np.float32),
            }
        )
    global _LAST_IN_MAPS
    _LAST_IN_MAPS = in_maps
    res = run_bass_kernel_spmd(nc, in_maps, core_ids=list(range(8)))
    x_out = np.empty((B, S, H), np.float32)
    lg_out = np.empty((B, S), np.float32)
    for c in range(8):
        b, j = c // 4, c % 4
        o = res.results[c]["out"]
        x_out[b, CH * j : CH * (j + 1)] = o[0:CH]
        x_out[b, CH * (7 - j) : CH * (8 - j)] = o[CH:512]
        if j == 0:
            lg_out[b] = res.results[c]["lg"]
    return x_out, lg_out
